# revision 43
# baseline (speedup 1.0000x reference)
"""MultiHeadLinearAttention Trainium2 kernel (8 NeuronCores, SPMD).

Sharding: core c handles batch b = c//2, head-group g = c%2 (4 of 8 heads,
i.e. feature slice F = [256g, 256g+256) of the 512 projection features).
Each core computes k/v/q projections restricted to its head-group, the
per-head linear-attention state over the full 8192-token sequence, and a
partial output in FEATURE-major layout outT[D, S] = (attn_F @ out_w[:, F].T).T.
The host transposes, sums the two partials per batch, and adds out_b.
No cross-core collectives are needed.

Math per head h (matches the fp32 jax reference):
  proj(x)  = silu(x@w1.T + b1) * (x@w2.T + b2)
  phi(x)   = elu(x) + 1 = max(x+1, exp(min(x, 0)))
  kv[d,e]  = sum_s phi_k[s,d] v[s,e]        (64x64 per head)
  ksum[d]  = sum_s phi_k[s,d]
  attn[s,e]= (sum_d phi_q[s,d] kv[d,e]) / (sum_d phi_q[s,d] ksum[d])
  out      = attn @ out_w.T + out_b

All matmul operands are bf16 (rel err ~5e-3 end to end, gate is 2e-2).
Denominators are computed into a [128, 512] PSUM tile prefilled with 1.0
(den ~1e5, so +1 is negligible) so one reciprocal_approx_fast covers all
heads without NaN lanes; the reciprocal rows are broadcast to the pair's
128 partitions with a fp32r sel-matmul.

Scalar-engine activations are batched per 2048-token chunk (all Silu,
then one wide Exp) to avoid activation-table reloads (Silu and Exp live
in different tables; each switch costs ~1.3us).
"""
import sys
sys.path.insert(0, '/opt/trn_rl_repo')

import numpy as np
import ml_dtypes
import concourse.bass as bass
import concourse.mybir as mybir
import concourse.tile as tile
from concourse.bass import ts, ds
from concourse.bass_utils import run_bass_kernel_spmd

F32 = mybir.dt.float32
F32R = mybir.dt.float32r
BF16 = mybir.dt.bfloat16
AF = mybir.ActivationFunctionType
OP = mybir.AluOpType

B, S, D = 4, 8192, 512
NH, DK = 8, 64
FG = 256            # features per head-group (4 heads = 2 pairs)
P = 128
CHUNK = 2048        # tokens per streamed/DMA chunk
NCHUNK = S // CHUNK          # 4
SUBT = CHUNK // P            # 16 subtiles of 128 tokens per chunk
QC = 512                     # phase-2 inner chunk (matmul moving max)
NQC = CHUNK // QC            # 4


def _split_waits(nc, limit=1):
    """walrus here rejects >1 embedded sync-wait per instruction; move extras
    onto same-engine NoOps immediately before (program order preserves
    semantics)."""
    uid = 0
    for f in nc.m.functions:
        for blk in f.blocks:
            new = []
            for ins in blk.instructions:
                si = ins.sync_info
                if si is not None and si.on_wait is not None and len(si.on_wait) > limit:
                    waits = list(si.on_wait)
                    head, keep = waits[:-limit], waits[-limit:]
                    for w in head:
                        nop = mybir.InstNoOp(
                            name=f"wsplit_{uid}", ins=[], outs=[],
                            sync_info=mybir.SyncInfo(on_wait=[w], on_update=[]))
                        uid += 1
                        nop.engine = ins.engine
                        new.append(nop)
                    ins.sync_info = mybir.SyncInfo(
                        on_wait=keep, on_update=list(si.on_update or []))
                new.append(ins)
            blk.instructions = new


def build_nc():
    nc = bass.Bass()

    # --- DRAM I/O (per-core data supplied via in_maps) ---
    xkT = nc.dram_tensor("xkT", [D, S], BF16, kind="ExternalInput")
    xvT = nc.dram_tensor("xvT", [D, S], BF16, kind="ExternalInput")
    xqT = nc.dram_tensor("xqT", [D, S], BF16, kind="ExternalInput")
    wk12T = nc.dram_tensor("wk12T", [D, 2 * FG], BF16, kind="ExternalInput")
    wv12T = nc.dram_tensor("wv12T", [D, 2 * FG], BF16, kind="ExternalInput")
    wq1T = nc.dram_tensor("wq1T", [D, FG], BF16, kind="ExternalInput")
    wq2T = nc.dram_tensor("wq2T", [D, FG], BF16, kind="ExternalInput")
    bk12p = nc.dram_tensor("bk12p", [P, 2 * FG], BF16, kind="ExternalInput")
    bv12p = nc.dram_tensor("bv12p", [P, 2 * FG], BF16, kind="ExternalInput")
    bq1 = nc.dram_tensor("bq1", [P, 2], F32, kind="ExternalInput")
    bq2 = nc.dram_tensor("bq2", [P, 2], F32, kind="ExternalInput")
    woT = nc.dram_tensor("woT", [FG, D], BF16, kind="ExternalInput")
    e0 = nc.dram_tensor("e0", [P, P], BF16, kind="ExternalInput")      # row0=1
    ones_col = nc.dram_tensor("ones_col", [P, 2], BF16, kind="ExternalInput")
    ones512 = nc.dram_tensor("ones512", [P, QC], BF16, kind="ExternalInput")
    sel01 = nc.dram_tensor("sel01", [P, 2 * P], F32R, kind="ExternalInput")
    bdz = nc.dram_tensor("bdz", [P, 2 * P], BF16, kind="ExternalInput")
    dkz = nc.dram_tensor("dkz", [P, 4], BF16, kind="ExternalInput")
    outT = nc.dram_tensor("outT", [D, S], BF16, kind="ExternalOutput")

    xkT_r = xkT.rearrange("(ko p) t -> p ko t", p=P)   # [128, 4, 8192]
    xvT_r = xvT.rearrange("(ko p) t -> p ko t", p=P)
    xqT_r = xqT.rearrange("(ko p) t -> p ko t", p=P)
    wk12T_r = wk12T.rearrange("(ko p) o -> p ko o", p=P)   # [128, 4, 512]
    wv12T_r = wv12T.rearrange("(ko p) o -> p ko o", p=P)
    wq1T_r = wq1T.rearrange("(ko p) o -> p ko o", p=P)     # [128, 4, 256]
    wq2T_r = wq2T.rearrange("(ko p) o -> p ko o", p=P)
    woT_r = woT.rearrange("(eo p) o -> p eo o", p=P)       # [128, 2, 512]
    outT_r = outT.rearrange("(mo p) t -> p mo t", p=P)     # [128, 4, 8192]

    with tile.TileContext(nc) as tc:
        with tc.tile_pool(name="const", bufs=1) as cpool:
            # Resident weights / constants
            wk_sb = cpool.tile([P, 4, 2 * FG], BF16)
            wv_sb = cpool.tile([P, 4, 2 * FG], BF16)
            wq1_sb = cpool.tile([P, 4, FG], BF16)
            wq2_sb = cpool.tile([P, 4, FG], BF16)
            wo_sb = cpool.tile([P, 2, D], BF16)
            bk_sb = cpool.tile([P, 2 * FG], BF16)
            bv_sb = cpool.tile([P, 2 * FG], BF16)
            bq1_sb = cpool.tile([P, 2], F32)
            bq2_sb = cpool.tile([P, 2], F32)
            e0_sb = cpool.tile([P, P], BF16)
            ones_sb = cpool.tile([P, 2], BF16)
            ones512_sb = cpool.tile([P, QC], BF16)
            sel_sb = cpool.tile([P, 2, P], F32R)
            nc.scalar.dma_start(e0_sb[:], e0[:])
            nc.scalar.dma_start(bk_sb[:], bk12p[:])
            nc.scalar.dma_start(bv_sb[:], bv12p[:])
            nc.scalar.dma_start(ones_sb[:], ones_col[:])
            nc.scalar.dma_start(ones512_sb[:], ones512[:])

            # Per-head-pair numerator/denominator lhsT built at phase boundary
            bd_sb = cpool.tile([P, 2, P], BF16)      # blockdiag kv per pair
            dk_sb = cpool.tile([P, 2, 2], BF16)      # ksum columns per pair

            # ---------------- Phase 1: k/v projections + state ----------------
            ctx_iop2 = tc.tile_pool(name="p2_io", bufs=2)
            iop2 = ctx_iop2.__enter__()
            with tc.tile_pool(name="p1_io", bufs=2) as iop, \
                 tc.tile_pool(name="p1_sb", bufs=3) as sbp, \
                 tc.tile_pool(name="p1_wide", bufs=2) as wpool, \
                 tc.tile_pool(name="p1_tmp", bufs=1) as tpool, \
                 tc.tile_pool(name="p1_ps", bufs=5, space="PSUM") as psp, \
                 tc.tile_pool(name="p1_st", bufs=1, space="PSUM") as stp:

                state_ps = stp.tile([P, 260], F32)   # kv pair0 | kv pair1 | ksum cols

                # State accumulation: ONE bank holds 4 regions (kv p0/p1,
                # ksum p0/p1). start=True clears has_written for the WHOLE
                # bank, so only the very first state matmul may use it; the
                # other regions' first matmuls overwrite (bits cleared) and
                # set their own bits, after which everything accumulates.
                # ksum is folded into the kv matmul: vprojw carries two extra
                # all-ones columns per pair, so out cols [130p+128, 130p+130)
                # accumulate sum_t phi_k (both cols identical; the dk copies
                # use col 0 for rows 0:64 and col 1 for rows 64:128).
                def emit_state(c, nsub, is_last, phikw, vprojw):
                    for s in range(nsub):
                        first = (c == 0 and s == 0)
                        last = (is_last and s == nsub - 1)
                        for p in range(2):
                            nc.tensor.matmul(state_ps[:, ds(130 * p, 130)],
                                             phikw[:, s, ts(p, P)],
                                             vprojw[:, s, ds(130 * p, 130)],
                                             start=(first and p == 0), stop=last,
                                             skip_group_check=True)

                # Small first chunk so the PE starts before the bulk DMA
                # lands; small last chunk so the exposed phi/state tail at the
                # phase boundary is short.
                P1_CHUNKS = [(0, 512), (512, 512), (1024, 1024), (2048, 2048),
                             (4096, 2048), (6144, 1536), (7680, 512)]
                pending = None
                qT_tiles = {}
                for c, (off, csz) in enumerate(P1_CHUNKS):
                    nsub = csz // P
                    last_chunk = (c == len(P1_CHUNKS) - 1)
                    # Split each chunk's loads across the sync and gpsimd DGE
                    # queues: a single queue sustains only ~130GB/s serially,
                    # which starves the PE mid-phase.
                    kT_c = iop.tile([P, 4, CHUNK], BF16, tag="kT")
                    vT_c = iop.tile([P, 4, CHUNK], BF16, tag="vT")
                    k_eng = [nc.sync, nc.scalar, nc.gpsimd, nc.sync]
                    v_eng = [nc.gpsimd, nc.sync, nc.scalar, nc.gpsimd]
                    for ki in range(4):
                        k_eng[ki].dma_start(kT_c[:, ki, :csz], xkT_r[:, ki, ds(off, csz)])
                        if c == 0:
                            nc.scalar.dma_start(wk_sb[:, ki, :], wk12T_r[:, ki, :])
                    for ki in range(4):
                        v_eng[ki].dma_start(vT_c[:, ki, :csz], xvT_r[:, ki, ds(off, csz)])
                        if c == 0:
                            nc.scalar.dma_start(wv_sb[:, ki, :], wv12T_r[:, ki, :])
                    if c == 1:
                        # phase-2 weights: load early, behind phase-1 traffic
                        nc.scalar.dma_start(wq1_sb[:], wq1T_r[:])
                        nc.scalar.dma_start(wq2_sb[:], wq2T_r[:])
                        nc.scalar.dma_start(wo_sb[:], woT_r[:])
                        nc.scalar.dma_start(bq1_sb[:], bq1[:])
                        nc.scalar.dma_start(bq2_sb[:], bq2[:])
                        nc.scalar.dma_start(sel_sb[:], sel01.rearrange("p (g m) -> p g m", m=P))
                    if last_chunk:
                        # prefetch phase-2's first q chunk behind phase-1 tail
                        qT0 = iop2.tile([P, 4, CHUNK], BF16, tag="qT")
                        qT_tiles[0] = qT0
                        for ki in range(4):
                            nc.scalar.dma_start(qT0[:, ki, :], xqT_r[:, ki, ds(0, CHUNK)])

                    kprojw = wpool.tile([P, SUBT, FG], BF16, tag="kprojw")
                    vprojw = wpool.tile([P, SUBT, 260], BF16, tag="vprojw")
                    mnkw = tpool.tile([P, SUBT, FG], BF16, tag="mnkw")
                    exkw = tpool.tile([P, SUBT, FG], BF16, tag="exkw")
                    phikw = wpool.tile([P, SUBT, FG], BF16, tag="phikw")
                    for p in range(2):
                        nc.gpsimd.memset(vprojw[:, :, ds(130 * p + P, 2)], 1.0)

                    for s in range(nsub):
                        tok = ds(s * P, P)
                        psk = psp.tile([P, 2 * FG], F32, tag="proj")
                        psv = psp.tile([P, 2 * FG], F32, tag="proj")
                        nc.tensor.matmul(psk[:], e0_sb[:], bk_sb[:], start=True, stop=False)
                        for ki in range(4):
                            nc.tensor.matmul(psk[:], kT_c[:, ki, tok], wk_sb[:, ki, :],
                                             start=False, stop=(ki == 3))
                        nc.tensor.matmul(psv[:], e0_sb[:], bv_sb[:], start=True, stop=False)
                        for ki in range(4):
                            nc.tensor.matmul(psv[:], vT_c[:, ki, tok], wv_sb[:, ki, :],
                                             start=False, stop=(ki == 3))
                        # k: silu(a1) * a2 ; v: silu(a1) * a2  (biases via e0)
                        silk = sbp.tile([P, FG], F32, tag="silk")
                        nc.scalar.activation(silk[:], psk[:, :FG], AF.Silu)
                        silv = sbp.tile([P, FG], F32, tag="silv")
                        nc.scalar.activation(silv[:], psv[:, :FG], AF.Silu)
                        nc.vector.tensor_tensor(kprojw[:, s, :], psk[:, FG:], silk[:], OP.mult)
                        for p in range(2):
                            nc.vector.tensor_tensor(
                                vprojw[:, s, ds(130 * p, P)],
                                psv[:, ds(FG + P * p, P)], silv[:, ds(P * p, P)],
                                OP.mult)

                    # Software pipeline: the previous chunk's state matmuls go
                    # behind this chunk's projection matmuls so the PE never
                    # waits on the scalar/vector phi chain.
                    if pending is not None:
                        emit_state(*pending)

                    # Wide phi pipeline: one exp per chunk (2 table loads/chunk)
                    nc.vector.tensor_scalar_min(mnkw[:, :nsub, :], kprojw[:, :nsub, :], 0.0)
                    nc.scalar.activation(exkw[:, :nsub, :], mnkw[:, :nsub, :], AF.Exp)
                    nc.vector.scalar_tensor_tensor(
                        phikw[:, :nsub, :], kprojw[:, :nsub, :], 1.0,
                        exkw[:, :nsub, :], OP.add, OP.max)
                    pending = (c, nsub, last_chunk, phikw, vprojw)

                emit_state(*pending)

                # --- phase boundary: build bd (blockdiag kv) and dk (ksum cols)
                nc.scalar.dma_start(bd_sb[:], bdz.rearrange("p (g m) -> p g m", m=P))
                nc.scalar.dma_start(dk_sb[:], dkz.rearrange("p (g m) -> p g m", m=2))
                for p in range(2):
                    nc.vector.tensor_copy(bd_sb[0:64, p, 0:64],
                                          state_ps[0:64, ds(130 * p, 64)])
                    nc.vector.tensor_copy(bd_sb[64:P, p, 64:P],
                                          state_ps[64:P, ds(130 * p + 64, 64)])
                    nc.vector.tensor_copy(dk_sb[0:64, p, 0:1],
                                          state_ps[0:64, ds(130 * p + P, 1)])
                    nc.vector.tensor_copy(dk_sb[64:P, p, 1:2],
                                          state_ps[64:P, ds(130 * p + P + 1, 1)])

            # ---------------- Phase 2: q projections + attention + out -------
            with tc.tile_pool(name="p2_sb", bufs=3) as sbp2, \
                 tc.tile_pool(name="p2_att", bufs=4) as attp2, \
                 tc.tile_pool(name="p2_wide", bufs=2) as wpool2, \
                 tc.tile_pool(name="p2_tmp", bufs=1) as tpool2, \
                 tc.tile_pool(name="p2_ps", bufs=2, space="PSUM") as psp2, \
                 tc.tile_pool(name="p2_ps_mm", bufs=3, space="PSUM") as psb2, \
                 tc.tile_pool(name="p2_ps_dn", bufs=1, space="PSUM") as psd2, \
                 tc.tile_pool(name="p2_ps_po", bufs=2, space="PSUM") as pso2:

                # out projection, FEATURE-major: po[m, t] so the DMA out has
                # 4KB-contiguous lines (host transposes).
                def emit_po(tok, atts, obw):
                    for m in range(4):
                        po = pso2.tile([P, QC], F32, tag="po")
                        nc.tensor.matmul(po[:], wo_sb[:, 0, ts(m, P)], atts[0][:],
                                         start=True, stop=False)
                        nc.tensor.matmul(po[:], wo_sb[:, 1, ts(m, P)], atts[1][:],
                                         start=False, stop=True)
                        if m % 2 == 0:
                            nc.scalar.copy(obw[:, m, tok], po[:])
                        else:
                            nc.vector.tensor_copy(obw[:, m, tok], po[:])

                def emit_attention(coff, nqc, phiqw, obw):
                    # Each qchunk's out-projection is emitted behind the next
                    # qchunk's num/rb matmuls (its att inputs are ready then).
                    pend_po = None
                    for qc in range(nqc):
                        tok = ds(qc * QC, QC)
                        # denominators, packed in one PSUM bank prefilled with
                        # 1.0 (den ~1e5; +1 negligible, keeps reciprocal sane
                        # on unused lanes). Pair p rows land at partition 32p.
                        dn = psd2.tile([P, QC], F32, tag="dn")
                        nc.tensor.matmul(dn[:], e0_sb[:], ones512_sb[:],
                                         start=True, stop=False, skip_group_check=True)
                        for p in range(2):
                            nc.tensor.matmul(dn[ds(32 * p, 2), :], dk_sb[:, p, :],
                                             phiqw[:, p, tok],
                                             start=False, stop=(p == 1),
                                             skip_group_check=True)
                        # rcp = exp(-ln(dn)): Ln and Exp share an activation
                        # table (with the wide phi Exp too). Vector reciprocal
                        # would be ~6.5ns/elem and serialize each qchunk.
                        lndn = sbp2.tile([P, QC], F32, tag="lndn")
                        nc.scalar.activation(lndn[:], dn[:], AF.Ln)
                        rcpr = attp2.tile([P, QC], F32R, tag="rcpr")
                        nc.scalar.activation(rcpr[:], lndn[:], AF.Exp, scale=-1.0)
                        atts = []
                        for p in range(2):
                            nump = psb2.tile([P, QC], F32, tag="mm")
                            nc.tensor.matmul(nump[:], bd_sb[:, p, :], phiqw[:, p, tok],
                                             start=True, stop=True)
                            rb = psb2.tile([P, QC], F32, tag="mm")
                            nc.tensor.matmul(rb[:], sel_sb[:, p, :], rcpr[:],
                                             start=True, stop=True)
                            rbs = sbp2.tile([P, QC], F32, tag="rbs")
                            nc.scalar.copy(rbs[:], rb[:])
                            att = attp2.tile([P, QC], BF16, tag="att")
                            nc.vector.tensor_tensor(att[:], nump[:], rbs[:], OP.mult)
                            atts.append(att)
                        if pend_po is not None:
                            emit_po(*pend_po)
                        pend_po = (tok, atts, obw)
                    emit_po(*pend_po)

                    for m in range(4):
                        nc.gpsimd.dma_start(outT_r[:, m, ds(coff, nqc * QC)],
                                            obw[:, m, :nqc * QC])

                # Small last chunk so the exposed attention tail is short.
                P2_CHUNKS = [(0, 2048), (2048, 2048), (4096, 2048),
                             (6144, 1536), (7680, 512)]
                pending2 = None
                for c, (off, csz) in enumerate(P2_CHUNKS):
                    nqc = csz // QC
                    if c in qT_tiles:
                        qT_c = qT_tiles[c]
                    else:
                        qT_c = iop2.tile([P, 4, CHUNK], BF16, tag="qT")
                        q_eng = [nc.sync, nc.sync, nc.scalar, nc.scalar]
                        for ki in range(4):
                            q_eng[ki].dma_start(qT_c[:, ki, :csz], xqT_r[:, ki, ds(off, csz)])

                    qpw = wpool2.tile([P, 2, CHUNK], BF16, tag="qpw")
                    mnqw = tpool2.tile([P, 2, CHUNK], BF16, tag="mnqw")
                    exqw = tpool2.tile([P, 2, CHUNK], BF16, tag="exqw")
                    phiqw = wpool2.tile([P, 2, CHUNK], BF16, tag="phiqw")
                    obw = wpool2.tile([P, 4, CHUNK], BF16, tag="obw")

                    # q projections for the whole chunk (scalar runs Silu only)
                    for qc in range(nqc):
                        tok = ds(qc * QC, QC)
                        for m in range(2):
                            ps1 = psp2.tile([P, QC], F32, tag="qproj")
                            ps2 = psp2.tile([P, QC], F32, tag="qproj")
                            for ki in range(4):
                                nc.tensor.matmul(ps1[:], wq1_sb[:, ki, ts(m, P)],
                                                 qT_c[:, ki, tok], start=(ki == 0), stop=(ki == 3))
                            for ki in range(4):
                                nc.tensor.matmul(ps2[:], wq2_sb[:, ki, ts(m, P)],
                                                 qT_c[:, ki, tok], start=(ki == 0), stop=(ki == 3))
                            sil = sbp2.tile([P, QC], F32, tag="sil")
                            nc.scalar.activation(sil[:], ps1[:], AF.Silu,
                                                 bias=bq1_sb[:, ds(m, 1)], scale=1.0)
                            nc.vector.scalar_tensor_tensor(
                                qpw[:, m, tok], ps2[:], bq2_sb[:, ds(m, 1)], sil[:],
                                OP.add, OP.mult)

                    # Software pipeline: previous chunk's attention matmuls go
                    # behind this chunk's q-projection matmuls.
                    if pending2 is not None:
                        emit_attention(*pending2)

                    # Wide phi pipeline: one exp per chunk
                    nc.vector.tensor_scalar_min(mnqw[:, :, :csz], qpw[:, :, :csz], 0.0)
                    nc.scalar.activation(exqw[:, :, :csz], mnqw[:, :, :csz], AF.Exp)
                    nc.vector.scalar_tensor_tensor(
                        phiqw[:, :, :csz], qpw[:, :, :csz], 1.0,
                        exqw[:, :, :csz], OP.add, OP.max)
                    pending2 = (off, nqc, phiqw, obw)

                emit_attention(*pending2)

            ctx_iop2.__exit__(None, None, None)

    _split_waits(nc)
    return nc


_NC_CACHE = None


def _get_nc():
    global _NC_CACHE
    if _NC_CACHE is None:
        _NC_CACHE = build_nc()
    return _NC_CACHE


def _prep_in_maps(inputs):
    return _build_in_maps(
        inputs["query"], inputs["key"], inputs["value"],
        inputs["q_w1"], inputs["q_w2"], inputs["k_w1"], inputs["k_w2"],
        inputs["v_w1"], inputs["v_w2"], inputs["out_w"],
        inputs["q_b1"], inputs["q_b2"], inputs["k_b1"], inputs["k_b2"],
        inputs["v_b1"], inputs["v_b2"])


def _build_in_maps(query, key, value,
                   q_w1, q_w2, k_w1, k_w2, v_w1, v_w2, out_w,
                   q_b1, q_b2, k_b1, k_b2, v_b1, v_b2):
    bf = ml_dtypes.bfloat16
    query = np.asarray(query, dtype=np.float32)
    key = np.asarray(key, dtype=np.float32)
    value = np.asarray(value, dtype=np.float32)

    e0 = np.zeros((P, P), bf); e0[0, :] = 1.0
    ones_col = np.ones((P, 2), bf)
    ones512 = np.ones((P, QC), bf)
    # sel01[k, (p, m)]: pair p, reciprocal row 32p+j -> partitions 64j..64j+63
    sel01 = np.zeros((P, 2, P), np.float32)
    for p in range(2):
        sel01[32 * p + 0, p, 0:64] = 1.0
        sel01[32 * p + 1, p, 64:128] = 1.0
    sel01 = sel01.reshape(P, 2 * P)
    bdz = np.zeros((P, 2 * P), bf)
    dkz = np.zeros((P, 4), bf)

    in_maps = []
    for c in range(8):
        b, g = c // 2, c % 2
        Fs = slice(FG * g, FG * (g + 1))
        bk12p = np.zeros((P, 2 * FG), bf)
        bk12p[0] = np.concatenate(
            [np.asarray(k_b1)[Fs], np.asarray(k_b2)[Fs]]).astype(bf)
        bv12p = np.zeros((P, 2 * FG), bf)
        bv12p[0] = np.concatenate(
            [np.asarray(v_b1)[Fs], np.asarray(v_b2)[Fs]]).astype(bf)
        in_maps.append({
            "xkT": np.ascontiguousarray(key[b].T).astype(bf),
            "xvT": np.ascontiguousarray(value[b].T).astype(bf),
            "xqT": np.ascontiguousarray(query[b].T).astype(bf),
            "wk12T": np.ascontiguousarray(np.concatenate(
                [np.asarray(k_w1)[Fs].T, np.asarray(k_w2)[Fs].T], axis=1)).astype(bf),
            "wv12T": np.ascontiguousarray(np.concatenate(
                [np.asarray(v_w1)[Fs].T, np.asarray(v_w2)[Fs].T], axis=1)).astype(bf),
            "wq1T": np.ascontiguousarray(np.asarray(q_w1)[Fs].T).astype(bf),
            "wq2T": np.ascontiguousarray(np.asarray(q_w2)[Fs].T).astype(bf),
            "bk12p": bk12p,
            "bv12p": bv12p,
            "bq1": np.ascontiguousarray(np.asarray(q_b1)[Fs].reshape(2, P).T.astype(np.float32)),
            "bq2": np.ascontiguousarray(np.asarray(q_b2)[Fs].reshape(2, P).T.astype(np.float32)),
            "woT": np.ascontiguousarray(np.asarray(out_w)[:, Fs].T).astype(bf),
            "e0": e0, "ones_col": ones_col, "ones512": ones512,
            "sel01": sel01, "bdz": bdz, "dkz": dkz,
        })
    return in_maps


def kernel(query, key, value,
           q_w1, q_w2, k_w1, k_w2, v_w1, v_w2, out_w,
           q_b1, q_b2, k_b1, k_b2, v_b1, v_b2, out_b):
    in_maps = _build_in_maps(query, key, value,
                             q_w1, q_w2, k_w1, k_w2, v_w1, v_w2, out_w,
                             q_b1, q_b2, k_b1, k_b2, v_b1, v_b2)
    nc = _get_nc()
    res = run_bass_kernel_spmd(nc, in_maps, core_ids=list(range(8)))
    ob = np.asarray(out_b, dtype=np.float32)
    out = np.empty((B, S, D), np.float32)
    for b in range(B):
        acc = (res.results[2 * b]["outT"].astype(np.float32)
               + res.results[2 * b + 1]["outT"].astype(np.float32))
        out[b] = acc.T + ob
    return out


# revision 45
# speedup vs baseline: 1.1959x; 1.1959x over previous
"""MultiHeadLinearAttention Trainium2 kernel (8 NeuronCores, SPMD).

Sharding: core c handles batch b = c//2, head-group g = c%2 (4 of 8 heads,
i.e. feature slice F = [256g, 256g+256) of the 512 projection features).
Each core computes k/v/q projections restricted to its head-group, the
per-head linear-attention state over the full 8192-token sequence, and a
partial output in FEATURE-major layout outT[D, S] = (attn_F @ out_w[:, F].T).T.
The host transposes, sums the two partials per batch, and adds out_b.
No cross-core collectives are needed.

Math per head h (matches the fp32 jax reference):
  proj(x)  = silu(x@w1.T + b1) * (x@w2.T + b2)
  phi(x)   = elu(x) + 1 = max(x+1, exp(min(x, 0)))
  kv[d,e]  = sum_s phi_k[s,d] v[s,e]        (64x64 per head)
  ksum[d]  = sum_s phi_k[s,d]
  attn[s,e]= (sum_d phi_q[s,d] kv[d,e]) / (sum_d phi_q[s,d] ksum[d])
  out      = attn @ out_w.T + out_b

All matmul operands are bf16 (rel err ~5e-3 end to end, gate is 2e-2).
Denominators are computed into a [128, 512] PSUM tile prefilled with 1.0
(den ~1e5, so +1 is negligible) so one reciprocal_approx_fast covers all
heads without NaN lanes; the reciprocal rows are broadcast to the pair's
128 partitions with a fp32r sel-matmul.

Scalar-engine activations are batched per 2048-token chunk (all Silu,
then one wide Exp) to avoid activation-table reloads (Silu and Exp live
in different tables; each switch costs ~1.3us).
"""
import sys
sys.path.insert(0, '/opt/trn_rl_repo')

import numpy as np
import ml_dtypes
import concourse.bass as bass
import concourse.mybir as mybir
import concourse.tile as tile
from concourse.bass import ts, ds
from concourse.bass_utils import run_bass_kernel_spmd

F32 = mybir.dt.float32
F32R = mybir.dt.float32r
BF16 = mybir.dt.bfloat16
AF = mybir.ActivationFunctionType
OP = mybir.AluOpType

B, S, D = 4, 8192, 512
NH, DK = 8, 64
FG = 256            # features per head-group (4 heads = 2 pairs)
P = 128
CHUNK = 2048        # tokens per streamed/DMA chunk
NCHUNK = S // CHUNK          # 4
SUBT = CHUNK // P            # 16 subtiles of 128 tokens per chunk
QC = 512                     # phase-2 inner chunk (matmul moving max)
NQC = CHUNK // QC            # 4


def _split_waits(nc, limit=1):
    """walrus here rejects >1 embedded sync-wait per instruction; move extras
    onto same-engine NoOps immediately before (program order preserves
    semantics)."""
    uid = 0
    for f in nc.m.functions:
        for blk in f.blocks:
            new = []
            for ins in blk.instructions:
                si = ins.sync_info
                if si is not None and si.on_wait is not None and len(si.on_wait) > limit:
                    waits = list(si.on_wait)
                    head, keep = waits[:-limit], waits[-limit:]
                    for w in head:
                        nop = mybir.InstNoOp(
                            name=f"wsplit_{uid}", ins=[], outs=[],
                            sync_info=mybir.SyncInfo(on_wait=[w], on_update=[]))
                        uid += 1
                        nop.engine = ins.engine
                        new.append(nop)
                    ins.sync_info = mybir.SyncInfo(
                        on_wait=keep, on_update=list(si.on_update or []))
                new.append(ins)
            blk.instructions = new


def build_nc():
    nc = bass.Bass()

    # --- DRAM I/O (per-core data supplied via in_maps) ---
    xkT = nc.dram_tensor("xkT", [D, S], BF16, kind="ExternalInput")
    xvT = nc.dram_tensor("xvT", [D, S], BF16, kind="ExternalInput")
    xqT = nc.dram_tensor("xqT", [D, S], BF16, kind="ExternalInput")
    wk12T = nc.dram_tensor("wk12T", [D, 2 * FG], BF16, kind="ExternalInput")
    wv12T = nc.dram_tensor("wv12T", [D, 2 * FG], BF16, kind="ExternalInput")
    wq1T = nc.dram_tensor("wq1T", [D, FG], BF16, kind="ExternalInput")
    wq2T = nc.dram_tensor("wq2T", [D, FG], BF16, kind="ExternalInput")
    bk12p = nc.dram_tensor("bk12p", [P, 2 * FG], BF16, kind="ExternalInput")
    bv12p = nc.dram_tensor("bv12p", [P, 2 * FG], BF16, kind="ExternalInput")
    bq1 = nc.dram_tensor("bq1", [P, 2], F32, kind="ExternalInput")
    bq2 = nc.dram_tensor("bq2", [P, 2], F32, kind="ExternalInput")
    woT = nc.dram_tensor("woT", [FG, D], BF16, kind="ExternalInput")
    e0 = nc.dram_tensor("e0", [P, P], BF16, kind="ExternalInput")      # row0=1
    ones_col = nc.dram_tensor("ones_col", [P, 2], BF16, kind="ExternalInput")
    ones512 = nc.dram_tensor("ones512", [P, QC], BF16, kind="ExternalInput")
    sel01 = nc.dram_tensor("sel01", [P, 2 * P], F32R, kind="ExternalInput")
    bdz = nc.dram_tensor("bdz", [P, 2 * P], BF16, kind="ExternalInput")
    dkz = nc.dram_tensor("dkz", [P, 4], BF16, kind="ExternalInput")
    outT = nc.dram_tensor("outT", [D, S], BF16, kind="ExternalOutput")

    xkT_r = xkT.rearrange("(ko p) t -> p ko t", p=P)   # [128, 4, 8192]
    xvT_r = xvT.rearrange("(ko p) t -> p ko t", p=P)
    xqT_r = xqT.rearrange("(ko p) t -> p ko t", p=P)
    wk12T_r = wk12T.rearrange("(ko p) o -> p ko o", p=P)   # [128, 4, 512]
    wv12T_r = wv12T.rearrange("(ko p) o -> p ko o", p=P)
    wq1T_r = wq1T.rearrange("(ko p) o -> p ko o", p=P)     # [128, 4, 256]
    wq2T_r = wq2T.rearrange("(ko p) o -> p ko o", p=P)
    woT_r = woT.rearrange("(eo p) o -> p eo o", p=P)       # [128, 2, 512]
    outT_r = outT.rearrange("(mo p) t -> p mo t", p=P)     # [128, 4, 8192]

    with tile.TileContext(nc) as tc:
        with tc.tile_pool(name="const", bufs=1) as cpool:
            # Resident weights / constants
            wk_sb = cpool.tile([P, 4, 2 * FG], BF16)
            wv_sb = cpool.tile([P, 4, 2 * FG], BF16)
            wq1_sb = cpool.tile([P, 4, FG], BF16)
            wq2_sb = cpool.tile([P, 4, FG], BF16)
            wo_sb = cpool.tile([P, 2, D], BF16)
            bk_sb = cpool.tile([P, 2 * FG], BF16)
            bv_sb = cpool.tile([P, 2 * FG], BF16)
            bq1_sb = cpool.tile([P, 2], F32)
            bq2_sb = cpool.tile([P, 2], F32)
            e0_sb = cpool.tile([P, P], BF16)
            ones_sb = cpool.tile([P, 2], BF16)
            ones512_sb = cpool.tile([P, QC], BF16)
            sel_sb = cpool.tile([P, 2, P], F32R)
            nc.scalar.dma_start(e0_sb[:], e0[:])
            nc.scalar.dma_start(bk_sb[:], bk12p[:])
            nc.scalar.dma_start(bv_sb[:], bv12p[:])
            nc.scalar.dma_start(ones_sb[:], ones_col[:])
            nc.scalar.dma_start(ones512_sb[:], ones512[:])

            # Per-head-pair numerator/denominator lhsT built at phase boundary
            bd_sb = cpool.tile([P, 2, P], BF16)      # blockdiag kv per pair
            dk_sb = cpool.tile([P, 2, 2], BF16)      # ksum columns per pair

            # ---------------- Phase 1: k/v projections + state ----------------
            ctx_iop2 = tc.tile_pool(name="p2_io", bufs=2)
            iop2 = ctx_iop2.__enter__()
            with tc.tile_pool(name="p1_io", bufs=2) as iop, \
                 tc.tile_pool(name="p1_sb", bufs=3) as sbp, \
                 tc.tile_pool(name="p1_wide", bufs=2) as wpool, \
                 tc.tile_pool(name="p1_tmp", bufs=1) as tpool, \
                 tc.tile_pool(name="p1_ps", bufs=6, space="PSUM") as psp, \
                 tc.tile_pool(name="p1_st", bufs=1, space="PSUM") as stp:

                state_ps = stp.tile([P, 260], F32)   # kv pair0 | kv pair1 | ksum cols

                # State accumulation: ONE bank holds 4 regions (kv p0/p1,
                # ksum p0/p1). start=True clears has_written for the WHOLE
                # bank, so only the very first state matmul may use it; the
                # other regions' first matmuls overwrite (bits cleared) and
                # set their own bits, after which everything accumulates.
                # ksum is folded into the kv matmul: vprojw carries two extra
                # all-ones columns per pair, so out cols [130p+128, 130p+130)
                # accumulate sum_t phi_k (both cols identical; the dk copies
                # use col 0 for rows 0:64 and col 1 for rows 64:128).
                def emit_state(c, nsub, is_last, phikw, vprojw):
                    for s in range(nsub):
                        first = (c == 0 and s == 0)
                        last = (is_last and s == nsub - 1)
                        for p in range(2):
                            nc.tensor.matmul(state_ps[:, ds(130 * p, 130)],
                                             phikw[:, s, ts(p, P)],
                                             vprojw[:, s, ds(130 * p, 130)],
                                             start=(first and p == 0), stop=last,
                                             skip_group_check=True)

                # Small first chunk so the PE starts before the bulk DMA
                # lands; small last chunk so the exposed phi/state tail at the
                # phase boundary is short.
                P1_CHUNKS = [(0, 512), (512, 512), (1024, 1024), (2048, 2048),
                             (4096, 2048), (6144, 1536), (7680, 512)]
                pending = None
                qT_tiles = {}
                for c, (off, csz) in enumerate(P1_CHUNKS):
                    nsub = csz // P
                    last_chunk = (c == len(P1_CHUNKS) - 1)
                    # Split each chunk's loads across the sync and gpsimd DGE
                    # queues: a single queue sustains only ~130GB/s serially,
                    # which starves the PE mid-phase.
                    kT_c = iop.tile([P, 4, CHUNK], BF16, tag="kT")
                    vT_c = iop.tile([P, 4, CHUNK], BF16, tag="vT")
                    k_eng = [nc.sync, nc.sync, nc.gpsimd, nc.gpsimd]
                    v_eng = [nc.gpsimd, nc.gpsimd, nc.sync, nc.sync]
                    for ki in range(4):
                        k_eng[ki].dma_start(kT_c[:, ki, :csz], xkT_r[:, ki, ds(off, csz)])
                        if c == 0:
                            nc.scalar.dma_start(wk_sb[:, ki, :], wk12T_r[:, ki, :])
                    for ki in range(4):
                        v_eng[ki].dma_start(vT_c[:, ki, :csz], xvT_r[:, ki, ds(off, csz)])
                        if c == 0:
                            nc.scalar.dma_start(wv_sb[:, ki, :], wv12T_r[:, ki, :])
                    if c == 1:
                        # phase-2 weights: load early, behind phase-1 traffic
                        nc.scalar.dma_start(wq1_sb[:], wq1T_r[:])
                        nc.scalar.dma_start(wq2_sb[:], wq2T_r[:])
                        nc.scalar.dma_start(wo_sb[:], woT_r[:])
                        nc.scalar.dma_start(bq1_sb[:], bq1[:])
                        nc.scalar.dma_start(bq2_sb[:], bq2[:])
                        nc.scalar.dma_start(sel_sb[:], sel01.rearrange("p (g m) -> p g m", m=P))
                    if last_chunk:
                        # prefetch phase-2's first q chunk behind phase-1 tail
                        qT0 = iop2.tile([P, 4, CHUNK], BF16, tag="qT")
                        qT_tiles[0] = qT0
                        for ki in range(4):
                            nc.scalar.dma_start(qT0[:, ki, :], xqT_r[:, ki, ds(0, CHUNK)])

                    kprojw = wpool.tile([P, SUBT, FG], BF16, tag="kprojw")
                    vprojw = wpool.tile([P, SUBT, 260], BF16, tag="vprojw")
                    mnkw = tpool.tile([P, SUBT, FG], BF16, tag="mnkw")
                    exkw = tpool.tile([P, SUBT, FG], BF16, tag="exkw")
                    phikw = wpool.tile([P, SUBT, FG], BF16, tag="phikw")
                    for p in range(2):
                        nc.gpsimd.memset(vprojw[:, :, ds(130 * p + P, 2)], 1.0)

                    for s in range(nsub):
                        tok = ds(s * P, P)
                        psk = psp.tile([P, 2 * FG], F32, tag="proj")
                        psv = psp.tile([P, 2 * FG], F32, tag="proj")
                        nc.tensor.matmul(psk[:], e0_sb[:], bk_sb[:], start=True, stop=False)
                        for ki in range(4):
                            nc.tensor.matmul(psk[:], kT_c[:, ki, tok], wk_sb[:, ki, :],
                                             start=False, stop=(ki == 3))
                        nc.tensor.matmul(psv[:], e0_sb[:], bv_sb[:], start=True, stop=False)
                        for ki in range(4):
                            nc.tensor.matmul(psv[:], vT_c[:, ki, tok], wv_sb[:, ki, :],
                                             start=False, stop=(ki == 3))
                        # k: silu(a1) * a2 ; v: silu(a1) * a2  (biases via e0)
                        silk = sbp.tile([P, FG], F32, tag="silk")
                        nc.scalar.activation(silk[:], psk[:, :FG], AF.Silu)
                        silv = sbp.tile([P, FG], F32, tag="silv")
                        nc.scalar.activation(silv[:], psv[:, :FG], AF.Silu)
                        nc.vector.tensor_tensor(kprojw[:, s, :], psk[:, FG:], silk[:], OP.mult)
                        for p in range(2):
                            nc.vector.tensor_tensor(
                                vprojw[:, s, ds(130 * p, P)],
                                psv[:, ds(FG + P * p, P)], silv[:, ds(P * p, P)],
                                OP.mult)

                    # Software pipeline: the previous chunk's state matmuls go
                    # behind this chunk's projection matmuls so the PE never
                    # waits on the scalar/vector phi chain.
                    if pending is not None:
                        emit_state(*pending)

                    # Wide phi pipeline: one exp per chunk (2 table loads/chunk)
                    nc.vector.tensor_scalar_min(mnkw[:, :nsub, :], kprojw[:, :nsub, :], 0.0)
                    nc.scalar.activation(exkw[:, :nsub, :], mnkw[:, :nsub, :], AF.Exp)
                    nc.vector.scalar_tensor_tensor(
                        phikw[:, :nsub, :], kprojw[:, :nsub, :], 1.0,
                        exkw[:, :nsub, :], OP.add, OP.max)
                    pending = (c, nsub, last_chunk, phikw, vprojw)

                emit_state(*pending)

                # --- phase boundary: build bd (blockdiag kv) and dk (ksum cols)
                nc.scalar.dma_start(bd_sb[:], bdz.rearrange("p (g m) -> p g m", m=P))
                nc.scalar.dma_start(dk_sb[:], dkz.rearrange("p (g m) -> p g m", m=2))
                for p in range(2):
                    nc.vector.tensor_copy(bd_sb[0:64, p, 0:64],
                                          state_ps[0:64, ds(130 * p, 64)])
                    nc.vector.tensor_copy(bd_sb[64:P, p, 64:P],
                                          state_ps[64:P, ds(130 * p + 64, 64)])
                    nc.vector.tensor_copy(dk_sb[0:64, p, 0:1],
                                          state_ps[0:64, ds(130 * p + P, 1)])
                    nc.vector.tensor_copy(dk_sb[64:P, p, 1:2],
                                          state_ps[64:P, ds(130 * p + P + 1, 1)])

            # ---------------- Phase 2: q projections + attention + out -------
            with tc.tile_pool(name="p2_sb", bufs=3) as sbp2, \
                 tc.tile_pool(name="p2_att", bufs=4) as attp2, \
                 tc.tile_pool(name="p2_wide", bufs=2) as wpool2, \
                 tc.tile_pool(name="p2_tmp", bufs=1) as tpool2, \
                 tc.tile_pool(name="p2_ps", bufs=2, space="PSUM") as psp2, \
                 tc.tile_pool(name="p2_ps_mm", bufs=3, space="PSUM") as psb2, \
                 tc.tile_pool(name="p2_ps_dn", bufs=1, space="PSUM") as psd2, \
                 tc.tile_pool(name="p2_ps_po", bufs=2, space="PSUM") as pso2:

                # out projection, FEATURE-major: po[m, t] so the DMA out has
                # 4KB-contiguous lines (host transposes).
                def emit_po(tok, atts, obw):
                    for m in range(4):
                        po = pso2.tile([P, QC], F32, tag="po")
                        nc.tensor.matmul(po[:], wo_sb[:, 0, ts(m, P)], atts[0][:],
                                         start=True, stop=False)
                        nc.tensor.matmul(po[:], wo_sb[:, 1, ts(m, P)], atts[1][:],
                                         start=False, stop=True)
                        if m % 2 == 0:
                            nc.scalar.copy(obw[:, m, tok], po[:])
                        else:
                            nc.vector.tensor_copy(obw[:, m, tok], po[:])

                def emit_attention(coff, nqc, phiqw, obw):
                    # Each qchunk's out-projection is emitted behind the next
                    # qchunk's num/rb matmuls (its att inputs are ready then).
                    pend_po = None
                    for qc in range(nqc):
                        tok = ds(qc * QC, QC)
                        # denominators, packed in one PSUM bank prefilled with
                        # 1.0 (den ~1e5; +1 negligible, keeps reciprocal sane
                        # on unused lanes). Pair p rows land at partition 32p.
                        dn = psd2.tile([P, QC], F32, tag="dn")
                        nc.tensor.matmul(dn[:], e0_sb[:], ones512_sb[:],
                                         start=True, stop=False, skip_group_check=True)
                        for p in range(2):
                            nc.tensor.matmul(dn[ds(32 * p, 2), :], dk_sb[:, p, :],
                                             phiqw[:, p, tok],
                                             start=False, stop=(p == 1),
                                             skip_group_check=True)
                        # rcp = exp(-ln(dn)): Ln and Exp share an activation
                        # table (with the wide phi Exp too). Vector reciprocal
                        # would be ~6.5ns/elem and serialize each qchunk.
                        lndn = sbp2.tile([P, QC], F32, tag="lndn")
                        nc.scalar.activation(lndn[:], dn[:], AF.Ln)
                        rcpr = attp2.tile([P, QC], F32R, tag="rcpr")
                        nc.scalar.activation(rcpr[:], lndn[:], AF.Exp, scale=-1.0)
                        atts = []
                        for p in range(2):
                            nump = psb2.tile([P, QC], F32, tag="mm")
                            nc.tensor.matmul(nump[:], bd_sb[:, p, :], phiqw[:, p, tok],
                                             start=True, stop=True)
                            rb = psb2.tile([P, QC], F32, tag="mm")
                            nc.tensor.matmul(rb[:], sel_sb[:, p, :], rcpr[:],
                                             start=True, stop=True)
                            rbs = sbp2.tile([P, QC], F32, tag="rbs")
                            nc.scalar.copy(rbs[:], rb[:])
                            att = attp2.tile([P, QC], BF16, tag="att")
                            nc.vector.tensor_tensor(att[:], nump[:], rbs[:], OP.mult)
                            atts.append(att)
                        if pend_po is not None:
                            emit_po(*pend_po)
                        pend_po = (tok, atts, obw)
                    emit_po(*pend_po)

                    for m in range(4):
                        nc.gpsimd.dma_start(outT_r[:, m, ds(coff, nqc * QC)],
                                            obw[:, m, :nqc * QC])

                # Small last chunk so the exposed attention tail is short.
                P2_CHUNKS = [(0, 2048), (2048, 2048), (4096, 2048),
                             (6144, 1536), (7680, 512)]
                pending2 = None
                for c, (off, csz) in enumerate(P2_CHUNKS):
                    nqc = csz // QC
                    if c in qT_tiles:
                        qT_c = qT_tiles[c]
                    else:
                        qT_c = iop2.tile([P, 4, CHUNK], BF16, tag="qT")
                        q_eng = [nc.sync, nc.sync, nc.scalar, nc.scalar]
                        for ki in range(4):
                            q_eng[ki].dma_start(qT_c[:, ki, :csz], xqT_r[:, ki, ds(off, csz)])

                    qpw = wpool2.tile([P, 2, CHUNK], BF16, tag="qpw")
                    mnqw = tpool2.tile([P, 2, CHUNK], BF16, tag="mnqw")
                    exqw = tpool2.tile([P, 2, CHUNK], BF16, tag="exqw")
                    phiqw = wpool2.tile([P, 2, CHUNK], BF16, tag="phiqw")
                    obw = wpool2.tile([P, 4, CHUNK], BF16, tag="obw")

                    # q projections for the whole chunk (scalar runs Silu only)
                    for qc in range(nqc):
                        tok = ds(qc * QC, QC)
                        for m in range(2):
                            ps1 = psp2.tile([P, QC], F32, tag="qproj")
                            ps2 = psp2.tile([P, QC], F32, tag="qproj")
                            for ki in range(4):
                                nc.tensor.matmul(ps1[:], wq1_sb[:, ki, ts(m, P)],
                                                 qT_c[:, ki, tok], start=(ki == 0), stop=(ki == 3))
                            for ki in range(4):
                                nc.tensor.matmul(ps2[:], wq2_sb[:, ki, ts(m, P)],
                                                 qT_c[:, ki, tok], start=(ki == 0), stop=(ki == 3))
                            sil = sbp2.tile([P, QC], F32, tag="sil")
                            nc.scalar.activation(sil[:], ps1[:], AF.Silu,
                                                 bias=bq1_sb[:, ds(m, 1)], scale=1.0)
                            nc.vector.scalar_tensor_tensor(
                                qpw[:, m, tok], ps2[:], bq2_sb[:, ds(m, 1)], sil[:],
                                OP.add, OP.mult)

                    # Software pipeline: previous chunk's attention matmuls go
                    # behind this chunk's q-projection matmuls.
                    if pending2 is not None:
                        emit_attention(*pending2)

                    # Wide phi pipeline: one exp per chunk
                    nc.vector.tensor_scalar_min(mnqw[:, :, :csz], qpw[:, :, :csz], 0.0)
                    nc.scalar.activation(exqw[:, :, :csz], mnqw[:, :, :csz], AF.Exp)
                    nc.vector.scalar_tensor_tensor(
                        phiqw[:, :, :csz], qpw[:, :, :csz], 1.0,
                        exqw[:, :, :csz], OP.add, OP.max)
                    pending2 = (off, nqc, phiqw, obw)

                emit_attention(*pending2)

            ctx_iop2.__exit__(None, None, None)

    _split_waits(nc)
    return nc


_NC_CACHE = None


def _get_nc():
    global _NC_CACHE
    if _NC_CACHE is None:
        _NC_CACHE = build_nc()
    return _NC_CACHE


def _prep_in_maps(inputs):
    return _build_in_maps(
        inputs["query"], inputs["key"], inputs["value"],
        inputs["q_w1"], inputs["q_w2"], inputs["k_w1"], inputs["k_w2"],
        inputs["v_w1"], inputs["v_w2"], inputs["out_w"],
        inputs["q_b1"], inputs["q_b2"], inputs["k_b1"], inputs["k_b2"],
        inputs["v_b1"], inputs["v_b2"])


def _build_in_maps(query, key, value,
                   q_w1, q_w2, k_w1, k_w2, v_w1, v_w2, out_w,
                   q_b1, q_b2, k_b1, k_b2, v_b1, v_b2):
    bf = ml_dtypes.bfloat16
    query = np.asarray(query, dtype=np.float32)
    key = np.asarray(key, dtype=np.float32)
    value = np.asarray(value, dtype=np.float32)

    e0 = np.zeros((P, P), bf); e0[0, :] = 1.0
    ones_col = np.ones((P, 2), bf)
    ones512 = np.ones((P, QC), bf)
    # sel01[k, (p, m)]: pair p, reciprocal row 32p+j -> partitions 64j..64j+63
    sel01 = np.zeros((P, 2, P), np.float32)
    for p in range(2):
        sel01[32 * p + 0, p, 0:64] = 1.0
        sel01[32 * p + 1, p, 64:128] = 1.0
    sel01 = sel01.reshape(P, 2 * P)
    bdz = np.zeros((P, 2 * P), bf)
    dkz = np.zeros((P, 4), bf)

    in_maps = []
    for c in range(8):
        b, g = c // 2, c % 2
        Fs = slice(FG * g, FG * (g + 1))
        bk12p = np.zeros((P, 2 * FG), bf)
        bk12p[0] = np.concatenate(
            [np.asarray(k_b1)[Fs], np.asarray(k_b2)[Fs]]).astype(bf)
        bv12p = np.zeros((P, 2 * FG), bf)
        bv12p[0] = np.concatenate(
            [np.asarray(v_b1)[Fs], np.asarray(v_b2)[Fs]]).astype(bf)
        in_maps.append({
            "xkT": np.ascontiguousarray(key[b].T).astype(bf),
            "xvT": np.ascontiguousarray(value[b].T).astype(bf),
            "xqT": np.ascontiguousarray(query[b].T).astype(bf),
            "wk12T": np.ascontiguousarray(np.concatenate(
                [np.asarray(k_w1)[Fs].T, np.asarray(k_w2)[Fs].T], axis=1)).astype(bf),
            "wv12T": np.ascontiguousarray(np.concatenate(
                [np.asarray(v_w1)[Fs].T, np.asarray(v_w2)[Fs].T], axis=1)).astype(bf),
            "wq1T": np.ascontiguousarray(np.asarray(q_w1)[Fs].T).astype(bf),
            "wq2T": np.ascontiguousarray(np.asarray(q_w2)[Fs].T).astype(bf),
            "bk12p": bk12p,
            "bv12p": bv12p,
            "bq1": np.ascontiguousarray(np.asarray(q_b1)[Fs].reshape(2, P).T.astype(np.float32)),
            "bq2": np.ascontiguousarray(np.asarray(q_b2)[Fs].reshape(2, P).T.astype(np.float32)),
            "woT": np.ascontiguousarray(np.asarray(out_w)[:, Fs].T).astype(bf),
            "e0": e0, "ones_col": ones_col, "ones512": ones512,
            "sel01": sel01, "bdz": bdz, "dkz": dkz,
        })
    return in_maps


def kernel(query, key, value,
           q_w1, q_w2, k_w1, k_w2, v_w1, v_w2, out_w,
           q_b1, q_b2, k_b1, k_b2, v_b1, v_b2, out_b):
    in_maps = _build_in_maps(query, key, value,
                             q_w1, q_w2, k_w1, k_w2, v_w1, v_w2, out_w,
                             q_b1, q_b2, k_b1, k_b2, v_b1, v_b2)
    nc = _get_nc()
    res = run_bass_kernel_spmd(nc, in_maps, core_ids=list(range(8)))
    ob = np.asarray(out_b, dtype=np.float32)
    out = np.empty((B, S, D), np.float32)
    for b in range(B):
        acc = (res.results[2 * b]["outT"].astype(np.float32)
               + res.results[2 * b + 1]["outT"].astype(np.float32))
        out[b] = acc.T + ob
    return out
